# revision 19
# baseline (speedup 1.0000x reference)
"""Trainium2 Bass kernel for the AutoAugment-controller LSTM step.

Reference computation (per batch row b):
    e      = embed_table[x[b]]                      # [H]
    gates  = e @ W_ih.T + b_ih + hx @ W_hh.T + b_hh # [4H]
    i,f,g,o = split(gates); i,f,o = sigmoid; g = tanh
    c' = f*cx + i*g ; h' = o*tanh(c')
    type_logit = h' @ W_type.T + b_type ; mag_logit = h' @ W_mag.T + b_mag

Strategy: data-parallel over the batch on 8 NeuronCores, everything kept in a
transposed [feature, batch] layout on-chip so no on-device transposes are
needed.  The embedding lookup is algebraically folded:
    e @ W_ih.T = onehot(x) @ (embed_table @ W_ih.T)
so the device computes fused = embed_table @ W_ih.T once (contraction over
vocab=176 instead of H=1024) and the per-batch embedding contribution becomes
a 176-deep matmul against a one-hot matrix built on-chip.  The biases ride
along as two extra fused-table rows whose one-hot rows are constant 1.
All matmuls run in float32r (full-rate fp32 mode of the PE).
"""

import ml_dtypes
import numpy as np

import bass_rust
import concourse.bass as bass
import concourse.mybir as mybir
import concourse.tile as tile
from concourse.bass_utils import run_bass_kernel_spmd

HID = 1024
VOCAB = 176
N_TYPE = 16
N_MAG = 11
NLOG = N_TYPE + N_MAG  # 27
G4 = 4 * HID
NCORES = 8
KT = HID // 128  # 8 contraction tiles
AF = mybir.ActivationFunctionType
F32 = mybir.dt.float32
F32R = mybir.dt.float32r
I32 = mybir.dt.int32
BF16 = mybir.dt.bfloat16
# stage-1 (fused-table) weight dtype: bf16 halves the W_ih stream but mixes
# bf16 FWL weight loads into the f32r matmul stream on the PE
S1_BF16 = False
S1_DT = BF16 if S1_BF16 else F32R
# shard the fused-table computation across the 8 cores + AllGather
USE_COLLECTIVE = False


def _legalize_waits(nc, max_waits=1):
    """walrus in this toolchain accepts at most one semaphore wait per
    instruction; split excess waits onto same-engine NoOps inserted before."""
    ctr = 0
    for f in nc.m.functions:
        for bb in f.blocks:
            insts = bb.instructions
            out = []
            for ins in insts:
                si = ins.sync_info
                if si is not None and len(si.on_wait) > max_waits:
                    waits = list(si.on_wait)
                    extra, keep = waits[:-max_waits], waits[-max_waits:]
                    for w in extra:
                        nop = mybir.InstNoOp(name=f"W-split-{ctr}", ins=[], outs=[])
                        ctr += 1
                        nop.engine = ins.engine
                        nop.sync_info = bass_rust.SyncInfo(on_wait=[w], on_update=[])
                        out.append(nop)
                    si.on_wait = keep
                out.append(ins)
            insts[:] = out


def _r(dtype):
    return F32R if dtype == F32 else dtype


def _mm(nc, out, lhsT, rhs, start, stop):
    nc.tensor.matmul(out, lhsT, rhs, start=start, stop=stop)


def _build(bc, legalize=True):
    """Build the single-core SPMD program for a per-core batch of `bc`."""
    assert bc % 512 == 0
    NB = bc // 512  # batch chunks of 512 (fp32 moving-operand limit)

    nc = bass.Bass(target_bir_lowering=False)

    xf = nc.dram_tensor("x_f32", [1, bc], F32, kind="ExternalInput")
    hxT = nc.dram_tensor("hxT", [HID, bc], F32R, kind="ExternalInput")
    cxT = nc.dram_tensor("cxT", [HID, bc], F32, kind="ExternalInput")
    etT = nc.dram_tensor("etT", [HID, VOCAB], S1_DT, kind="ExternalInput")
    # W_ih.T columns permuted identically to whhP (col = hi*512 + gi*128 + c)
    if USE_COLLECTIVE:
        # each core receives only its 512-column slice of the permuted W_ih.T
        wihS = nc.dram_tensor("wihS", [HID, 512], S1_DT, kind="ExternalInput")
    else:
        wihP = nc.dram_tensor("wihP", [HID, G4], S1_DT, kind="ExternalInput")
    # W_hh.T with columns permuted so that for each h-tile the four gate
    # column-blocks are adjacent: col = hi*512 + gi*128 + c
    whhP = nc.dram_tensor("whhP", [HID, G4], F32R, kind="ExternalInput")
    bih = nc.dram_tensor("bih", [1, G4], F32R, kind="ExternalInput")
    bhh = nc.dram_tensor("bhh", [1, G4], F32R, kind="ExternalInput")
    wtmT = nc.dram_tensor("wtmT", [HID, NLOG], F32R, kind="ExternalInput")
    btm = nc.dram_tensor("btm", [NLOG, 1], F32, kind="ExternalInput")

    hT_out = nc.dram_tensor("hT_out", [HID, bc], F32R, kind="ExternalOutput")
    cT_out = nc.dram_tensor("cT_out", [HID, bc], F32, kind="ExternalOutput")
    loT_out = nc.dram_tensor("loT_out", [NLOG, bc], F32, kind="ExternalOutput")

    with tile.TileContext(nc) as tc:
        with (
            tc.tile_pool(name="res", bufs=1) as res,
            tc.tile_pool(name="wih", bufs=2) as wih_pool,
            tc.tile_pool(name="whh", bufs=2) as whh_pool,
            tc.tile_pool(name="gates", bufs=8) as gates,
            tc.tile_pool(name="misc", bufs=2) as misc,
            tc.tile_pool(name="psum", bufs=6, space="PSUM") as psum,
            tc.tile_pool(name="psuml", bufs=NB, space="PSUM") as psuml,
        ):
            # ---- small constants / one-hot build --------------------------
            et_sb = res.tile([128, KT, VOCAB], S1_DT, tag="et_sb")
            nc.sync.dma_start(
                out=et_sb, in_=etT[:, :].rearrange("(k p) v -> p k v", p=128)
            )
            wtm_sb = res.tile([128, KT, NLOG], F32R, tag="wtm_sb")
            nc.sync.dma_start(
                out=wtm_sb, in_=wtmT[:, :].rearrange("(k p) v -> p k v", p=128)
            )
            btm_sb = res.tile([NLOG, 1], F32, tag="btm_sb")
            nc.sync.dma_start(out=btm_sb, in_=btm[:, :])

            xa = xf[:, :]
            xb0 = res.tile([128, bc], F32, tag="xb0")
            nc.sync.dma_start(
                out=xb0,
                in_=bass.AP(tensor=xa.tensor, offset=xa.offset, ap=[[0, 128], [1, bc]]),
            )
            iota0 = res.tile([128, 1], F32, tag="iota0")
            nc.gpsimd.iota(iota0, pattern=[[0, 1]], base=0, channel_multiplier=1,
                           allow_small_or_imprecise_dtypes=True)
            iota1 = res.tile([48, 1], F32, tag="iota1")
            nc.gpsimd.iota(iota1, pattern=[[0, 1]], base=128, channel_multiplier=1,
                           allow_small_or_imprecise_dtypes=True)
            # oh0 rows 0..127 <-> vocab ids 0..127.
            # oh1 rows 0..47 <-> vocab ids 128..175; rows 48..63 zero; rows
            # 64,65 constant 1 (bias rows; partition starts must be x32)
            oh0 = res.tile([128, bc], F32R, tag="oh0")
            nc.vector.tensor_scalar(oh0, xb0, iota0, None, op0=mybir.AluOpType.is_equal)
            oh1 = res.tile([66, bc], F32R, tag="oh1")
            nc.vector.memset(oh1[:, :].bitcast(F32), 0.0)
            nc.vector.tensor_scalar(
                oh1[0:48, :], xb0[0:48, :], iota1, None, op0=mybir.AluOpType.is_equal
            )
            nc.vector.memset(oh1[64:66, :].bitcast(F32), 1.0)

            # ---- fused table ----------------------------------------------
            fus0 = res.tile([128, KT, 512], F32R, tag="fus0")
            fus1 = res.tile([66, KT, 512], F32R, tag="fus1")
            nc.vector.memset(fus1[:, :, :].bitcast(F32), 0.0)
            nc.sync.dma_start(
                out=fus1[64:65, :, :], in_=bih[:, :].rearrange("o (h c) -> o h c", c=512)
            )
            nc.sync.dma_start(
                out=fus1[65:66, :, :], in_=bhh[:, :].rearrange("o (h c) -> o h c", c=512)
            )

            def s1_chunk(j):
                # fused-table columns in the same permuted order as whhP, so
                # chunk j provides exactly the columns main_hi(j) consumes
                w = wih_pool.tile([128, KT, 512], S1_DT, tag="wih")
                for k in range(KT):
                    nc.sync.dma_start(
                        out=w[:, k, :],
                        in_=wihP[k * 128 : (k + 1) * 128, j * 512 : (j + 1) * 512],
                    )
                for fus, voff, vp in ((fus0, 0, 128), (fus1, 128, 48)):
                    ps = psum.tile([128, 512], F32, tag="psum")
                    for k in range(KT):
                        nc.tensor.matmul(
                            ps[:vp, :],
                            et_sb[:, k, voff : voff + vp],
                            w[:, k, :],
                            start=(k == 0),
                            stop=(k == KT - 1),
                        )
                    nc.scalar.copy(fus[:vp, j, :], ps[:vp, :])

            def s1_collective(dram):
                # this core computes fused[:, my 512 cols] from its wihS slice,
                # then an AllGather assembles the full table on every core
                w = wih_pool.tile([128, KT, 512], S1_DT, tag="wih")
                for k in range(KT):
                    nc.sync.dma_start(
                        out=w[:, k, :], in_=wihS[k * 128 : (k + 1) * 128, :]
                    )
                loc = dram.tile([VOCAB, 512], F32R, tag="loc")
                gat = dram.tile([NCORES * VOCAB, 512], F32R, tag="gat", addr_space="Shared")
                for voff, vp in ((0, 128), (128, 48)):
                    ps = psum.tile([128, 512], F32, tag="psum")
                    for k in range(KT):
                        nc.tensor.matmul(
                            ps[:vp, :],
                            et_sb[:, k, voff : voff + vp],
                            w[:, k, :],
                            start=(k == 0),
                            stop=(k == KT - 1),
                        )
                    t = misc.tile([128, 512], F32R, tag="floc", name=f"floc{voff}")
                    nc.scalar.copy(t[:vp, :], ps[:vp, :])
                    nc.sync.dma_start(out=loc[voff : voff + vp, :], in_=t[:vp, :])
                nc.gpsimd.collective_compute(
                    "AllGather",
                    mybir.AluOpType.bypass,
                    replica_groups=[list(range(NCORES))],
                    ins=[loc.opt()],
                    outs=[gat.opt()],
                )
                gv = gat.rearrange("(h v) c -> v h c", v=VOCAB)
                nc.sync.dma_start(out=fus0[:, :, :], in_=gv[0:128, :, :])
                nc.sync.dma_start(out=fus1[0:48, :, :], in_=gv[128:176, :, :])

            # ---- main loop ------------------------------------------------
            hx_sb = res.tile([128, KT, bc], F32R, tag="hx_sb")
            lo_ps = [psuml.tile([128, 512], F32, tag="lops", name=f"lo_ps{n}") for n in range(NB)]
            GATE_FUNC = (AF.Sigmoid, AF.Sigmoid, AF.Tanh, AF.Sigmoid)  # i, f, g, o

            def main_hi(hi):
                wt = whh_pool.tile([128, KT, 512], F32R, tag="whh")
                for k in range(KT):
                    nc.sync.dma_start(
                        out=wt[:, k, :],
                        in_=whhP[k * 128 : (k + 1) * 128, hi * 512 : (hi + 1) * 512],
                    )
                for n in range(NB):
                    bs = n * 512
                    pss = [
                        psum.tile([128, 512], F32, tag="psum", name=f"ps{hi}_{n}_{gi}")
                        for gi in range(4)
                    ]
                    # k outer / gate inner: consecutive matmuls hit different
                    # PSUM banks, so each weight load hides under the previous
                    # bank's moving stream
                    for k in range(KT):
                        for gi in range(4):
                            _mm(
                                nc,
                                pss[gi],
                                wt[:, k, gi * 128 : (gi + 1) * 128],
                                hx_sb[:, k, bs : bs + 512],
                                start=(k == 0),
                                stop=False,
                            )
                    for gi in range(4):
                        fc = gi * 128
                        _mm(
                            nc,
                            pss[gi],
                            fus0[:, hi, fc : fc + 128],
                            oh0[:, bs : bs + 512],
                            start=False,
                            stop=False,
                        )
                        _mm(
                            nc,
                            pss[gi],
                            fus1[:, hi, fc : fc + 128],
                            oh1[:, bs : bs + 512],
                            start=False,
                            stop=True,
                        )
                    g_sb = []
                    for gi in range(4):
                        g = gates.tile([128, 512], F32, tag="gact")
                        nc.scalar.activation(g, pss[gi], GATE_FUNC[gi])
                        g_sb.append(g)
                    cx_t = misc.tile([128, 512], F32, tag="cx_t")
                    nc.sync.dma_start(
                        out=cx_t, in_=cxT[hi * 128 : (hi + 1) * 128, bs : bs + 512]
                    )
                    t1 = misc.tile([128, 512], F32, tag="tmp")
                    nc.vector.tensor_mul(t1, g_sb[0], g_sb[2])  # i*g
                    t2 = misc.tile([128, 512], F32, tag="tmp")
                    nc.vector.tensor_mul(t2, g_sb[1], cx_t)  # f*cx
                    c_new = misc.tile([128, 512], F32, tag="c_new")
                    nc.vector.tensor_add(c_new, t1, t2)
                    nc.sync.dma_start(
                        out=cT_out[hi * 128 : (hi + 1) * 128, bs : bs + 512], in_=c_new
                    )
                    tc_t = misc.tile([128, 512], F32, tag="tc_t")
                    nc.scalar.activation(tc_t, c_new, AF.Tanh)
                    ht_t = misc.tile([128, 512], F32R, tag="ht_t")
                    nc.vector.tensor_mul(ht_t, g_sb[3], tc_t)
                    nc.sync.dma_start(
                        out=hT_out[hi * 128 : (hi + 1) * 128, bs : bs + 512], in_=ht_t
                    )
                    # logits contribution of this h-tile (27-row accumulation)
                    nc.tensor.matmul(
                        lo_ps[n][:NLOG, :],
                        wtm_sb[:, hi, :],
                        ht_t,
                        start=(hi == 0),
                        stop=(hi == KT - 1),
                        skip_group_check=True,
                    )

            # ---- emission order -------------------------------------------
            if USE_COLLECTIVE:
                with tc.tile_pool(name="dram", bufs=1, space="DRAM") as dram:
                    s1_collective(dram)
                    for n in range(NB):
                        for k in range(KT):
                            nc.sync.dma_start(
                                out=hx_sb[:, k, n * 512 : (n + 1) * 512],
                                in_=hxT[
                                    k * 128 : (k + 1) * 128, n * 512 : (n + 1) * 512
                                ],
                            )
                    for hi in range(KT):
                        main_hi(hi)
            else:
                # stage-1 chunk j feeds main_hi(j); keep one chunk ahead so
                # the critical startup DMAs (wih0, hx n=0, whh0) come first
                s1_chunk(0)
                for n in range(NB):
                    for k in range(KT):
                        nc.sync.dma_start(
                            out=hx_sb[:, k, n * 512 : (n + 1) * 512],
                            in_=hxT[k * 128 : (k + 1) * 128, n * 512 : (n + 1) * 512],
                        )
                for hi in range(KT):
                    main_hi(hi)
                    if hi + 1 < KT:
                        s1_chunk(hi + 1)

            # ---- decoder logits evacuation --------------------------------
            for n in range(NB):
                bs = n * 512
                lo = misc.tile([NLOG, 512], F32, tag="lo")
                nc.scalar.activation(lo, lo_ps[n][:NLOG, :], AF.Identity, bias=btm_sb)
                nc.sync.dma_start(out=loT_out[:, bs : bs + 512], in_=lo)

    if legalize:
        _legalize_waits(nc)
    return nc


_NC_CACHE = {}


def _get_nc(bc):
    if bc not in _NC_CACHE:
        _NC_CACHE[bc] = _build(bc)
    return _NC_CACHE[bc]


def _permute_gate_cols(b):
    # same column permutation as whhP/wihP: col = hi*512 + gi*128 + c
    return np.ascontiguousarray(
        b.reshape(4, KT, 128).transpose(1, 0, 2).reshape(G4)[None, :]
    )


def _prep_shared(embed_table, W_ih, W_hh, b_ih, b_hh, W_type, b_type, W_mag, b_mag):
    f = np.float32
    s1dt = ml_dtypes.bfloat16 if S1_BF16 else f
    etT = np.ascontiguousarray(np.asarray(embed_table, f).T.astype(s1dt))
    wihT = np.asarray(W_ih, f).T  # [H, 4H], col = gi*1024 + hi*128 + c
    wihP = np.ascontiguousarray(
        wihT.reshape(HID, 4, KT, 128).transpose(0, 2, 1, 3).reshape(HID, G4).astype(s1dt)
    )
    whhT = np.asarray(W_hh, f).T
    whhP = np.ascontiguousarray(
        whhT.reshape(HID, 4, KT, 128).transpose(0, 2, 1, 3).reshape(HID, G4)
    )
    wtmT = np.ascontiguousarray(
        np.concatenate([np.asarray(W_type, f), np.asarray(W_mag, f)], axis=0).T
    )
    btm = np.ascontiguousarray(
        np.concatenate([np.asarray(b_type, f), np.asarray(b_mag, f)])[:, None]
    )
    return {
        "etT": etT,
        "_wihP_full": wihP,
        "whhP": whhP,
        "bih": _permute_gate_cols(np.asarray(b_ih, f)),
        "bhh": _permute_gate_cols(np.asarray(b_hh, f)),
        "wtmT": wtmT,
        "btm": btm,
    }


def make_in_maps(
    x,
    hx,
    cx,
    batch_size,
    embed_table,
    W_ih,
    W_hh,
    b_ih,
    b_hh,
    W_type,
    b_type,
    W_mag,
    b_mag,
):
    f = np.float32
    x = np.asarray(x)
    hx = np.asarray(hx, f)
    cx = np.asarray(cx, f)
    B = hx.shape[0]
    bc = B // NCORES

    shared = _prep_shared(
        embed_table, W_ih, W_hh, b_ih, b_hh, W_type, b_type, W_mag, b_mag
    )

    hxT = np.ascontiguousarray(hx.T)  # [H, B]
    cxT = np.ascontiguousarray(cx.T)
    x32 = np.asarray(x, np.int64).astype(np.int32)

    in_maps = []
    for c in range(NCORES):
        s = slice(c * bc, (c + 1) * bc)
        m = dict(shared)
        wf = m.pop("_wihP_full")
        if USE_COLLECTIVE:
            m["wihS"] = np.ascontiguousarray(wf[:, c * 512 : (c + 1) * 512])
        else:
            m["wihP"] = wf
        m["x_f32"] = np.ascontiguousarray(x32[s].astype(np.float32)[None, :])
        m["hxT"] = np.ascontiguousarray(hxT[:, s])
        m["cxT"] = np.ascontiguousarray(cxT[:, s])
        in_maps.append(m)
    return in_maps, bc


def run_spmd(in_maps, bc, **kw):
    nc = _get_nc(bc)
    return run_bass_kernel_spmd(nc, in_maps, list(range(NCORES)), **kw)


def postprocess(res):
    hT = np.concatenate([r["hT_out"] for r in res.results], axis=1)  # [H, B]
    cT = np.concatenate([r["cT_out"] for r in res.results], axis=1)
    loT = np.concatenate([r["loT_out"] for r in res.results], axis=1)  # [27, B]

    hx_new = np.ascontiguousarray(hT.T)
    cx_new = np.ascontiguousarray(cT.T)
    logits = loT.T  # [B, 27]
    type_logit = np.ascontiguousarray(logits[:, :N_TYPE])
    magnitude_logit = np.ascontiguousarray(logits[:, N_TYPE:])
    return (type_logit, magnitude_logit, hx_new, cx_new)


def kernel(**inputs):
    in_maps, bc = make_in_maps(**inputs)
    res = run_spmd(in_maps, bc)
    return postprocess(res)


# revision 20
# speedup vs baseline: 1.0270x; 1.0270x over previous
"""Trainium2 Bass kernel for the AutoAugment-controller LSTM step.

Reference computation (per batch row b):
    e      = embed_table[x[b]]                      # [H]
    gates  = e @ W_ih.T + b_ih + hx @ W_hh.T + b_hh # [4H]
    i,f,g,o = split(gates); i,f,o = sigmoid; g = tanh
    c' = f*cx + i*g ; h' = o*tanh(c')
    type_logit = h' @ W_type.T + b_type ; mag_logit = h' @ W_mag.T + b_mag

Strategy: data-parallel over the batch on 8 NeuronCores, everything kept in a
transposed [feature, batch] layout on-chip so no on-device transposes are
needed.  The embedding lookup is algebraically folded:
    e @ W_ih.T = onehot(x) @ (embed_table @ W_ih.T)
so the device computes fused = embed_table @ W_ih.T once (contraction over
vocab=176 instead of H=1024) and the per-batch embedding contribution becomes
a 176-deep matmul against a one-hot matrix built on-chip.  The biases ride
along as two extra fused-table rows whose one-hot rows are constant 1.
All matmuls run in float32r (full-rate fp32 mode of the PE).
"""

import ml_dtypes
import numpy as np

import bass_rust
import concourse.bass as bass
import concourse.mybir as mybir
import concourse.tile as tile
from concourse.bass_utils import run_bass_kernel_spmd

HID = 1024
VOCAB = 176
N_TYPE = 16
N_MAG = 11
NLOG = N_TYPE + N_MAG  # 27
G4 = 4 * HID
NCORES = 8
KT = HID // 128  # 8 contraction tiles
AF = mybir.ActivationFunctionType
F32 = mybir.dt.float32
F32R = mybir.dt.float32r
I32 = mybir.dt.int32
BF16 = mybir.dt.bfloat16
# stage-1 (fused-table) weight dtype: bf16 halves the W_ih stream but mixes
# bf16 FWL weight loads into the f32r matmul stream on the PE
S1_BF16 = False
S1_DT = BF16 if S1_BF16 else F32R
# shard the fused-table computation across the 8 cores + AllGather
USE_COLLECTIVE = False


def _legalize_waits(nc, max_waits=1):
    """walrus in this toolchain accepts at most one semaphore wait per
    instruction; split excess waits onto same-engine NoOps inserted before."""
    ctr = 0
    for f in nc.m.functions:
        for bb in f.blocks:
            insts = bb.instructions
            out = []
            for ins in insts:
                si = ins.sync_info
                if si is not None and len(si.on_wait) > max_waits:
                    waits = list(si.on_wait)
                    extra, keep = waits[:-max_waits], waits[-max_waits:]
                    for w in extra:
                        nop = mybir.InstNoOp(name=f"W-split-{ctr}", ins=[], outs=[])
                        ctr += 1
                        nop.engine = ins.engine
                        nop.sync_info = bass_rust.SyncInfo(on_wait=[w], on_update=[])
                        out.append(nop)
                    si.on_wait = keep
                out.append(ins)
            insts[:] = out


def _r(dtype):
    return F32R if dtype == F32 else dtype


def _mm(nc, out, lhsT, rhs, start, stop):
    nc.tensor.matmul(out, lhsT, rhs, start=start, stop=stop)


def _build(bc, legalize=True):
    """Build the single-core SPMD program for a per-core batch of `bc`."""
    assert bc % 512 == 0
    NB = bc // 512  # batch chunks of 512 (fp32 moving-operand limit)

    nc = bass.Bass(target_bir_lowering=False)

    xf = nc.dram_tensor("x_f32", [1, bc], F32, kind="ExternalInput")
    hxT = nc.dram_tensor("hxT", [HID, bc], F32R, kind="ExternalInput")
    cxT = nc.dram_tensor("cxT", [HID, bc], F32, kind="ExternalInput")
    etT = nc.dram_tensor("etT", [HID, VOCAB], S1_DT, kind="ExternalInput")
    # W_ih.T columns permuted identically to whhP (col = hi*512 + gi*128 + c)
    if USE_COLLECTIVE:
        # each core receives only its 512-column slice of the permuted W_ih.T
        wihS = nc.dram_tensor("wihS", [HID, 512], S1_DT, kind="ExternalInput")
    else:
        wihP = nc.dram_tensor("wihP", [HID, G4], S1_DT, kind="ExternalInput")
    # W_hh.T with columns permuted so that for each h-tile the four gate
    # column-blocks are adjacent: col = hi*512 + gi*128 + c
    whhP = nc.dram_tensor("whhP", [HID, G4], F32R, kind="ExternalInput")
    bih = nc.dram_tensor("bih", [1, G4], F32R, kind="ExternalInput")
    bhh = nc.dram_tensor("bhh", [1, G4], F32R, kind="ExternalInput")
    wtmT = nc.dram_tensor("wtmT", [HID, NLOG], F32R, kind="ExternalInput")
    btm = nc.dram_tensor("btm", [NLOG, 1], F32, kind="ExternalInput")

    hT_out = nc.dram_tensor("hT_out", [HID, bc], F32R, kind="ExternalOutput")
    cT_out = nc.dram_tensor("cT_out", [HID, bc], F32, kind="ExternalOutput")
    loT_out = nc.dram_tensor("loT_out", [NLOG, bc], F32, kind="ExternalOutput")

    with tile.TileContext(nc) as tc:
        with (
            tc.tile_pool(name="res", bufs=1) as res,
            tc.tile_pool(name="wih", bufs=2) as wih_pool,
            tc.tile_pool(name="whh", bufs=2) as whh_pool,
            tc.tile_pool(name="gates", bufs=8) as gates,
            tc.tile_pool(name="misc", bufs=2) as misc,
            tc.tile_pool(name="psum", bufs=6, space="PSUM") as psum,
            tc.tile_pool(name="psuml", bufs=NB, space="PSUM") as psuml,
        ):
            # ---- small constants / one-hot build --------------------------
            et_sb = res.tile([128, KT, VOCAB], S1_DT, tag="et_sb")
            nc.sync.dma_start(
                out=et_sb, in_=etT[:, :].rearrange("(k p) v -> p k v", p=128)
            )
            wtm_sb = res.tile([128, KT, NLOG], F32R, tag="wtm_sb")
            nc.sync.dma_start(
                out=wtm_sb, in_=wtmT[:, :].rearrange("(k p) v -> p k v", p=128)
            )
            btm_sb = res.tile([NLOG, 1], F32, tag="btm_sb")
            nc.sync.dma_start(out=btm_sb, in_=btm[:, :])

            xa = xf[:, :]
            xb0 = res.tile([128, bc], F32, tag="xb0")
            nc.sync.dma_start(
                out=xb0,
                in_=bass.AP(tensor=xa.tensor, offset=xa.offset, ap=[[0, 128], [1, bc]]),
            )
            iota0 = res.tile([128, 1], F32, tag="iota0")
            nc.gpsimd.iota(iota0, pattern=[[0, 1]], base=0, channel_multiplier=1,
                           allow_small_or_imprecise_dtypes=True)
            iota1 = res.tile([48, 1], F32, tag="iota1")
            nc.gpsimd.iota(iota1, pattern=[[0, 1]], base=128, channel_multiplier=1,
                           allow_small_or_imprecise_dtypes=True)
            # oh0 rows 0..127 <-> vocab ids 0..127.
            # oh1 rows 0..47 <-> vocab ids 128..175; rows 48..63 zero; rows
            # 64,65 constant 1 (bias rows; partition starts must be x32)
            oh0 = res.tile([128, bc], F32R, tag="oh0")
            nc.vector.tensor_scalar(oh0, xb0, iota0, None, op0=mybir.AluOpType.is_equal)
            oh1 = res.tile([66, bc], F32R, tag="oh1")
            nc.vector.memset(oh1[:, :].bitcast(F32), 0.0)
            nc.vector.tensor_scalar(
                oh1[0:48, :], xb0[0:48, :], iota1, None, op0=mybir.AluOpType.is_equal
            )
            nc.vector.memset(oh1[64:66, :].bitcast(F32), 1.0)

            # ---- fused table ----------------------------------------------
            fus0 = res.tile([128, KT, 512], F32R, tag="fus0")
            fus1 = res.tile([66, KT, 512], F32R, tag="fus1")
            nc.vector.memset(fus1[:, :, :].bitcast(F32), 0.0)
            nc.sync.dma_start(
                out=fus1[64:65, :, :], in_=bih[:, :].rearrange("o (h c) -> o h c", c=512)
            )
            nc.sync.dma_start(
                out=fus1[65:66, :, :], in_=bhh[:, :].rearrange("o (h c) -> o h c", c=512)
            )

            def s1_chunk(j):
                # fused-table columns in the same permuted order as whhP, so
                # chunk j provides exactly the columns main_hi(j) consumes
                w = wih_pool.tile([128, KT, 512], S1_DT, tag="wih")
                for k in range(KT):
                    nc.sync.dma_start(
                        out=w[:, k, :],
                        in_=wihP[k * 128 : (k + 1) * 128, j * 512 : (j + 1) * 512],
                    )
                for fus, voff, vp in ((fus0, 0, 128), (fus1, 128, 48)):
                    ps = psum.tile([128, 512], F32, tag="psum")
                    for k in range(KT):
                        nc.tensor.matmul(
                            ps[:vp, :],
                            et_sb[:, k, voff : voff + vp],
                            w[:, k, :],
                            start=(k == 0),
                            stop=(k == KT - 1),
                        )
                    nc.scalar.copy(fus[:vp, j, :], ps[:vp, :])

            def s1_collective(dram):
                # this core computes fused[:, my 512 cols] from its wihS slice,
                # then an AllGather assembles the full table on every core
                w = wih_pool.tile([128, KT, 512], S1_DT, tag="wih")
                for k in range(KT):
                    nc.sync.dma_start(
                        out=w[:, k, :], in_=wihS[k * 128 : (k + 1) * 128, :]
                    )
                loc = dram.tile([VOCAB, 512], F32R, tag="loc")
                gat = dram.tile([NCORES * VOCAB, 512], F32R, tag="gat", addr_space="Shared")
                for voff, vp in ((0, 128), (128, 48)):
                    ps = psum.tile([128, 512], F32, tag="psum")
                    for k in range(KT):
                        nc.tensor.matmul(
                            ps[:vp, :],
                            et_sb[:, k, voff : voff + vp],
                            w[:, k, :],
                            start=(k == 0),
                            stop=(k == KT - 1),
                        )
                    t = misc.tile([128, 512], F32R, tag="floc", name=f"floc{voff}")
                    nc.scalar.copy(t[:vp, :], ps[:vp, :])
                    nc.sync.dma_start(out=loc[voff : voff + vp, :], in_=t[:vp, :])
                nc.gpsimd.collective_compute(
                    "AllGather",
                    mybir.AluOpType.bypass,
                    replica_groups=[list(range(NCORES))],
                    ins=[loc.opt()],
                    outs=[gat.opt()],
                )
                gv = gat.rearrange("(h v) c -> v h c", v=VOCAB)
                nc.sync.dma_start(out=fus0[:, :, :], in_=gv[0:128, :, :])
                nc.sync.dma_start(out=fus1[0:48, :, :], in_=gv[128:176, :, :])

            # ---- main loop ------------------------------------------------
            hx_sb = res.tile([128, KT, bc], F32R, tag="hx_sb")
            lo_ps = [psuml.tile([128, 512], F32, tag="lops", name=f"lo_ps{n}") for n in range(NB)]
            GATE_FUNC = (AF.Sigmoid, AF.Sigmoid, AF.Tanh, AF.Sigmoid)  # i, f, g, o

            def load_whh(hi):
                wt = whh_pool.tile([128, KT, 512], F32R, tag="whh", name=f"wt{hi}")
                for k in range(KT):
                    nc.sync.dma_start(
                        out=wt[:, k, :],
                        in_=whhP[k * 128 : (k + 1) * 128, hi * 512 : (hi + 1) * 512],
                    )
                return wt

            def main_hi(hi, wt=None):
                if wt is None:
                    wt = load_whh(hi)
                for n in range(NB):
                    bs = n * 512
                    pss = [
                        psum.tile([128, 512], F32, tag="psum", name=f"ps{hi}_{n}_{gi}")
                        for gi in range(4)
                    ]
                    # k outer / gate inner: consecutive matmuls hit different
                    # PSUM banks, so each weight load hides under the previous
                    # bank's moving stream
                    for k in range(KT):
                        for gi in range(4):
                            _mm(
                                nc,
                                pss[gi],
                                wt[:, k, gi * 128 : (gi + 1) * 128],
                                hx_sb[:, k, bs : bs + 512],
                                start=(k == 0),
                                stop=False,
                            )
                    for gi in range(4):
                        fc = gi * 128
                        _mm(
                            nc,
                            pss[gi],
                            fus0[:, hi, fc : fc + 128],
                            oh0[:, bs : bs + 512],
                            start=False,
                            stop=False,
                        )
                        _mm(
                            nc,
                            pss[gi],
                            fus1[:, hi, fc : fc + 128],
                            oh1[:, bs : bs + 512],
                            start=False,
                            stop=True,
                        )
                    g_sb = []
                    for gi in range(4):
                        g = gates.tile([128, 512], F32, tag="gact")
                        nc.scalar.activation(g, pss[gi], GATE_FUNC[gi])
                        g_sb.append(g)
                    cx_t = misc.tile([128, 512], F32, tag="cx_t")
                    nc.sync.dma_start(
                        out=cx_t, in_=cxT[hi * 128 : (hi + 1) * 128, bs : bs + 512]
                    )
                    t1 = misc.tile([128, 512], F32, tag="tmp")
                    nc.vector.tensor_mul(t1, g_sb[0], g_sb[2])  # i*g
                    t2 = misc.tile([128, 512], F32, tag="tmp")
                    nc.vector.tensor_mul(t2, g_sb[1], cx_t)  # f*cx
                    c_new = misc.tile([128, 512], F32, tag="c_new")
                    nc.vector.tensor_add(c_new, t1, t2)
                    nc.sync.dma_start(
                        out=cT_out[hi * 128 : (hi + 1) * 128, bs : bs + 512], in_=c_new
                    )
                    tc_t = misc.tile([128, 512], F32, tag="tc_t")
                    nc.scalar.activation(tc_t, c_new, AF.Tanh)
                    ht_t = misc.tile([128, 512], F32R, tag="ht_t")
                    nc.vector.tensor_mul(ht_t, g_sb[3], tc_t)
                    nc.sync.dma_start(
                        out=hT_out[hi * 128 : (hi + 1) * 128, bs : bs + 512], in_=ht_t
                    )
                    # logits contribution of this h-tile (27-row accumulation)
                    nc.tensor.matmul(
                        lo_ps[n][:NLOG, :],
                        wtm_sb[:, hi, :],
                        ht_t,
                        start=(hi == 0),
                        stop=(hi == KT - 1),
                        skip_group_check=True,
                    )

            # ---- emission order -------------------------------------------
            if USE_COLLECTIVE:
                with tc.tile_pool(name="dram", bufs=1, space="DRAM") as dram:
                    s1_collective(dram)
                    for n in range(NB):
                        for k in range(KT):
                            nc.sync.dma_start(
                                out=hx_sb[:, k, n * 512 : (n + 1) * 512],
                                in_=hxT[
                                    k * 128 : (k + 1) * 128, n * 512 : (n + 1) * 512
                                ],
                            )
                    for hi in range(KT):
                        main_hi(hi)
            else:
                # stage-1 chunk j feeds main_hi(j); keep one chunk ahead so
                # the critical startup DMAs (wih0, hx n=0, whh0) come first
                s1_chunk(0)
                # critical-path first: hx n=0, then whh(0), then the rest
                for k in range(KT):
                    nc.sync.dma_start(
                        out=hx_sb[:, k, 0:512], in_=hxT[k * 128 : (k + 1) * 128, 0:512]
                    )
                wt0 = load_whh(0)
                for n in range(1, NB):
                    for k in range(KT):
                        nc.sync.dma_start(
                            out=hx_sb[:, k, n * 512 : (n + 1) * 512],
                            in_=hxT[k * 128 : (k + 1) * 128, n * 512 : (n + 1) * 512],
                        )
                for hi in range(KT):
                    main_hi(hi, wt=wt0 if hi == 0 else None)
                    if hi + 1 < KT:
                        s1_chunk(hi + 1)

            # ---- decoder logits evacuation --------------------------------
            for n in range(NB):
                bs = n * 512
                lo = misc.tile([NLOG, 512], F32, tag="lo")
                nc.scalar.activation(lo, lo_ps[n][:NLOG, :], AF.Identity, bias=btm_sb)
                nc.sync.dma_start(out=loT_out[:, bs : bs + 512], in_=lo)

    if legalize:
        _legalize_waits(nc)
    return nc


_NC_CACHE = {}


def _get_nc(bc):
    if bc not in _NC_CACHE:
        _NC_CACHE[bc] = _build(bc)
    return _NC_CACHE[bc]


def _permute_gate_cols(b):
    # same column permutation as whhP/wihP: col = hi*512 + gi*128 + c
    return np.ascontiguousarray(
        b.reshape(4, KT, 128).transpose(1, 0, 2).reshape(G4)[None, :]
    )


def _prep_shared(embed_table, W_ih, W_hh, b_ih, b_hh, W_type, b_type, W_mag, b_mag):
    f = np.float32
    s1dt = ml_dtypes.bfloat16 if S1_BF16 else f
    etT = np.ascontiguousarray(np.asarray(embed_table, f).T.astype(s1dt))
    wihT = np.asarray(W_ih, f).T  # [H, 4H], col = gi*1024 + hi*128 + c
    wihP = np.ascontiguousarray(
        wihT.reshape(HID, 4, KT, 128).transpose(0, 2, 1, 3).reshape(HID, G4).astype(s1dt)
    )
    whhT = np.asarray(W_hh, f).T
    whhP = np.ascontiguousarray(
        whhT.reshape(HID, 4, KT, 128).transpose(0, 2, 1, 3).reshape(HID, G4)
    )
    wtmT = np.ascontiguousarray(
        np.concatenate([np.asarray(W_type, f), np.asarray(W_mag, f)], axis=0).T
    )
    btm = np.ascontiguousarray(
        np.concatenate([np.asarray(b_type, f), np.asarray(b_mag, f)])[:, None]
    )
    return {
        "etT": etT,
        "_wihP_full": wihP,
        "whhP": whhP,
        "bih": _permute_gate_cols(np.asarray(b_ih, f)),
        "bhh": _permute_gate_cols(np.asarray(b_hh, f)),
        "wtmT": wtmT,
        "btm": btm,
    }


def make_in_maps(
    x,
    hx,
    cx,
    batch_size,
    embed_table,
    W_ih,
    W_hh,
    b_ih,
    b_hh,
    W_type,
    b_type,
    W_mag,
    b_mag,
):
    f = np.float32
    x = np.asarray(x)
    hx = np.asarray(hx, f)
    cx = np.asarray(cx, f)
    B = hx.shape[0]
    bc = B // NCORES

    shared = _prep_shared(
        embed_table, W_ih, W_hh, b_ih, b_hh, W_type, b_type, W_mag, b_mag
    )

    hxT = np.ascontiguousarray(hx.T)  # [H, B]
    cxT = np.ascontiguousarray(cx.T)
    x32 = np.asarray(x, np.int64).astype(np.int32)

    in_maps = []
    for c in range(NCORES):
        s = slice(c * bc, (c + 1) * bc)
        m = dict(shared)
        wf = m.pop("_wihP_full")
        if USE_COLLECTIVE:
            m["wihS"] = np.ascontiguousarray(wf[:, c * 512 : (c + 1) * 512])
        else:
            m["wihP"] = wf
        m["x_f32"] = np.ascontiguousarray(x32[s].astype(np.float32)[None, :])
        m["hxT"] = np.ascontiguousarray(hxT[:, s])
        m["cxT"] = np.ascontiguousarray(cxT[:, s])
        in_maps.append(m)
    return in_maps, bc


def run_spmd(in_maps, bc, **kw):
    nc = _get_nc(bc)
    return run_bass_kernel_spmd(nc, in_maps, list(range(NCORES)), **kw)


def postprocess(res):
    hT = np.concatenate([r["hT_out"] for r in res.results], axis=1)  # [H, B]
    cT = np.concatenate([r["cT_out"] for r in res.results], axis=1)
    loT = np.concatenate([r["loT_out"] for r in res.results], axis=1)  # [27, B]

    hx_new = np.ascontiguousarray(hT.T)
    cx_new = np.ascontiguousarray(cT.T)
    logits = loT.T  # [B, 27]
    type_logit = np.ascontiguousarray(logits[:, :N_TYPE])
    magnitude_logit = np.ascontiguousarray(logits[:, N_TYPE:])
    return (type_logit, magnitude_logit, hx_new, cx_new)


def kernel(**inputs):
    in_maps, bc = make_in_maps(**inputs)
    res = run_spmd(in_maps, bc)
    return postprocess(res)
